# revision 2
# baseline (speedup 1.0000x reference)
"""Multi-head attention (B=2, S=2048, D=1024, H=16, causal) on 8 Trainium2 cores.

Sharding: core c handles batch b = c // 4 and head group g = c % 4 (4 heads,
d_model column slice [256*g, 256*g+256)).  QKV projections are computed per
core against the full sequence of its batch; attention runs per head in a
"scores-transposed" [k, q] layout which makes every matmul operand land in
its natural layout (no on-device transposes of activations beyond the initial
DMA-xbar transpose of x); the output projection produces a per-core partial
[S, D] that the host sums over the 4 head-group cores of each batch.

All matmul operands are bf16 (host pre-casts); accumulation is fp32 in PSUM,
softmax runs in fp32 on the ACT engine (exp with the 1/sqrt(dh) scale fused).
The softmax denominator comes for free from 64 ones-columns appended to V:
P@V output rows 64:127 all hold the denominator, so the reciprocal is already
broadcast across partitions for the normalize multiply.
"""

import functools
import numpy as np
import ml_dtypes

import concourse.bass as bass
import concourse.bacc as bacc
import concourse.tile as tile
import concourse.mybir as mybir
from concourse.bass_utils import run_bass_kernel_spmd

dt = mybir.dt
F32 = dt.float32
BF16 = dt.bfloat16
AFT = mybir.ActivationFunctionType

B, S, D = 2, 2048, 1024
H, DH = 16, 64
NCORES = 8
GROUPS = NCORES // B            # 4 head-groups
HC = H // GROUPS                # 4 heads per core
C = HC * DH                     # 256 = per-core head-column slice
P = 128
DK = D // P                     # 8 d_in chunks
SB = 512                        # q-slab width
NSLAB = S // SB                 # 4
KT = S // P                     # 16 k tiles
SCALE = 1.0 / float(np.sqrt(DH))


def _build(mask_mode: str):
    """mask_mode: 'causal' | 'none' | 'generic'. Returns compiled Bacc."""
    assert mask_mode in ("causal", "none", "generic")
    nc = bacc.Bacc("TRN2", target_bir_lowering=False, debug=False)

    xq_d = nc.dram_tensor("xq", [S, D], BF16, kind="ExternalInput").ap()
    xk_d = nc.dram_tensor("xk", [S, D], BF16, kind="ExternalInput").ap()
    xv_d = nc.dram_tensor("xv", [S, D], BF16, kind="ExternalInput").ap()
    wqT_d = nc.dram_tensor("wqT", [P, DK, C], BF16, kind="ExternalInput").ap()
    wkT_d = nc.dram_tensor("wkT", [P, DK, C], BF16, kind="ExternalInput").ap()
    wvT_d = nc.dram_tensor("wvT", [P, DK, C], BF16, kind="ExternalInput").ap()
    woT_d = nc.dram_tensor("woT", [P, C // P, D], BF16, kind="ExternalInput").ap()
    bq_d = nc.dram_tensor("bq", [P, C // P], F32, kind="ExternalInput").ap()
    bk_d = nc.dram_tensor("bk", [P, C // P], F32, kind="ExternalInput").ap()
    bvb_d = nc.dram_tensor("bvb", [P, C], F32, kind="ExternalInput").ap()
    if mask_mode == "causal":
        strips_d = nc.dram_tensor("strips", [P, SB // P, SB], BF16,
                                  kind="ExternalInput").ap()
    if mask_mode == "generic":
        maskT_d = nc.dram_tensor("maskT", [S, S], BF16, kind="ExternalInput").ap()
    o_d = nc.dram_tensor("o", [S, D], F32, kind="ExternalOutput").ap()

    with tile.TileContext(nc) as tc:
        with (
            tc.tile_pool(name="consts", bufs=1) as consts,
            tc.tile_pool(name="xT", bufs=2) as xT_pool,
            tc.tile_pool(name="acts", bufs=1) as acts,
            tc.tile_pool(name="expT", bufs=2) as exp_pool,
            tc.tile_pool(name="stage", bufs=2) as stage,
            tc.tile_pool(name="pp", bufs=2, space="PSUM") as pp,
            tc.tile_pool(name="sps", bufs=2, space="PSUM") as sps,
            tc.tile_pool(name="otp", bufs=2, space="PSUM") as otp,
        ):
            # ---- constants
            wq_sb = consts.tile([P, DK, C], BF16)
            wk_sb = consts.tile([P, DK, C], BF16)
            wv_sb = consts.tile([P, DK, C], BF16)
            wo_sb = consts.tile([P, C // P, D], BF16)
            bq_sb = consts.tile([P, C // P], F32)
            bk_sb = consts.tile([P, C // P], F32)
            bvb_sb = consts.tile([P, C], F32)
            nc.sync.dma_start(wq_sb[:], wqT_d)
            nc.sync.dma_start(wk_sb[:], wkT_d)
            nc.sync.dma_start(wv_sb[:], wvT_d)
            nc.sync.dma_start(wo_sb[:], woT_d)
            nc.sync.dma_start(bq_sb[:], bq_d)
            nc.sync.dma_start(bk_sb[:], bk_d)
            nc.sync.dma_start(bvb_sb[:], bvb_d)
            if mask_mode == "causal":
                strips_sb = consts.tile([P, SB // P, SB], BF16)
                nc.sync.dma_start(strips_sb[:], strips_d)

            # ---- activations in SBUF
            qT_sb = acts.tile([P, C // P, S], BF16)       # [d_out, s] head-major
            kT_sb = acts.tile([P, C // P, S], BF16)
            headsT_sb = acts.tile([P, C // P, S], BF16)
            v_sb = acts.tile([P, KT, HC, P], BF16)        # cols 0:64 v, 64:128 ones

            nc.vector.memset(v_sb[:, :, :, DH:P], 1.0)

            # ---- x transposes (DMA xbar): xT [128 d_in, chunk, s]
            xqT = xT_pool.tile([P, DK, S], BF16, tag="xT", name="xqT")
            for o in range(DK):
                nc.sync.dma_start_transpose(xqT[:, o, :], xq_d[:, o * P:(o + 1) * P])
            xkT = xT_pool.tile([P, DK, S], BF16, tag="xT", name="xkT")
            for o in range(DK):
                nc.sync.dma_start_transpose(xkT[:, o, :], xk_d[:, o * P:(o + 1) * P])

            # ---- Q/K projections (weight-stationary): qT[dout, s]
            for (w_sb, b_sb, outT, xT) in ((wq_sb, bq_sb, qT_sb, xqT),
                                           (wk_sb, bk_sb, kT_sb, xkT)):
                for co in range(C // P):
                    for j in range(NSLAB):
                        ps = pp.tile([P, SB], F32, tag="pp", name="proj_ps")
                        for o in range(DK):
                            nc.tensor.matmul(
                                ps[:],
                                lhsT=w_sb[:, o, co * P:(co + 1) * P],
                                rhs=xT[:, o, j * SB:(j + 1) * SB],
                                start=(o == 0), stop=(o == DK - 1))
                        nc.vector.tensor_scalar_add(
                            outT[:, co, j * SB:(j + 1) * SB], ps[:],
                            b_sb[:, co:co + 1])

            # ---- V projection (x-stationary): v[s, head, dh] with ones cols
            xvT = xT_pool.tile([P, DK, S], BF16, tag="xT", name="xvT")
            for o in range(DK):
                nc.sync.dma_start_transpose(xvT[:, o, :], xv_d[:, o * P:(o + 1) * P])
            for st in range(KT):
                ps = pp.tile([P, SB], F32, tag="pp", name="vproj_ps")
                for o in range(DK):
                    nc.tensor.matmul(
                        ps[:, 0:C],
                        lhsT=xvT[:, o, st * P:(st + 1) * P],
                        rhs=wv_sb[:, o, :],
                        start=(o == 0), stop=(o == DK - 1))
                nc.vector.tensor_add(
                    v_sb[:, st, :, 0:DH],
                    ps[:, 0:C].rearrange("p (h d) -> p h d", h=HC),
                    bvb_sb[:].rearrange("p (h d) -> p h d", h=HC))

            # ---- attention, per head / q-slab, scores-transposed [k, q]
            for h in range(HC):
                hp = DH * (h % 2)          # partition base of this head
                hc = h // 2                # chunk index
                for j in range(NSLAB):
                    n_kt = 4 * (j + 1) if mask_mode == "causal" else KT
                    expT = exp_pool.tile([P, KT, SB], BF16, tag="expT", name="expT")
                    outp = otp.tile([P, SB], F32, tag="otp", name="outp")
                    for tb in range(0, n_kt, 2):
                        npair = min(2, n_kt - tb)
                        sp = sps.tile([P, 2, SB], F32, tag="sps", name="sp")
                        for d_ in range(npair):
                            t = tb + d_
                            nc.tensor.matmul(
                                sp[:, d_, :],
                                lhsT=kT_sb[hp:hp + DH, hc, t * P:(t + 1) * P],
                                rhs=qT_sb[hp:hp + DH, hc, j * SB:(j + 1) * SB],
                                start=True, stop=True)
                        nc.scalar.activation(
                            expT[:, tb:tb + npair, :], sp[:, 0:npair, :],
                            AFT.Exp, scale=SCALE)
                        for d_ in range(npair):
                            t = tb + d_
                            if mask_mode == "causal" and t >= 4 * j:
                                i = t - 4 * j
                                nc.vector.tensor_mul(
                                    expT[:, t, :], expT[:, t, :],
                                    strips_sb[:, i, :])
                            elif mask_mode == "generic":
                                m_sb = stage.tile([P, SB], BF16, tag="msk",
                                                  name="m_sb")
                                nc.sync.dma_start(
                                    m_sb[:],
                                    maskT_d[t * P:(t + 1) * P,
                                            j * SB:(j + 1) * SB])
                                nc.vector.tensor_mul(
                                    expT[:, t, :], expT[:, t, :], m_sb[:])
                            nc.tensor.matmul(
                                outp[:],
                                lhsT=v_sb[:, t, h, :],
                                rhs=expT[:, t, :],
                                start=(t == 0), stop=(t == n_kt - 1))
                    # normalize: rows 64:128 of outp are the denominator.
                    # reciprocal_approx_fast mis-reads when in/out partition
                    # bases differ, so compute it over all 128 rows (rows
                    # 0:64 are recip of the unnormalized output — unused).
                    recip = stage.tile([P, SB], F32, tag="recip", name="recip")
                    nc.vector.reciprocal_approx_fast(recip[:], outp[:])
                    nc.vector.tensor_mul(
                        headsT_sb[hp:hp + DH, hc, j * SB:(j + 1) * SB],
                        outp[0:DH, :], recip[DH:P, :])

            # ---- output projection (partial over this core's 256 columns)
            for st in range(KT):
                ob = stage.tile([P, D], F32, tag="ob", name="ob")
                for n2 in range(D // SB):
                    ps = pp.tile([P, SB], F32, tag="pp", name="o_ps")
                    for cc in range(C // P):
                        nc.tensor.matmul(
                            ps[:],
                            lhsT=headsT_sb[:, cc, st * P:(st + 1) * P],
                            rhs=wo_sb[:, cc, n2 * SB:(n2 + 1) * SB],
                            start=(cc == 0), stop=(cc == C // P - 1))
                    nc.any.tensor_copy(ob[:, n2 * SB:(n2 + 1) * SB], ps[:])
                nc.sync.dma_start(o_d[st * P:(st + 1) * P, :], ob[:])

    nc.compile()
    return nc


@functools.lru_cache(maxsize=4)
def _get(mask_mode: str):
    return _build(mask_mode)


def _bf16(a):
    return np.ascontiguousarray(a.astype(ml_dtypes.bfloat16))


def _detect_mask_mode(m):
    if (m == 1).all():
        return "none"
    idx = np.arange(m.shape[0])
    if np.array_equal(m != 0, idx[None, :] <= idx[:, None]):
        return "causal"
    return "generic"


def _strips():
    p = np.arange(P)[:, None]
    f = np.arange(SB)[None, :]
    s = np.stack([(p <= f - P * i) for i in range(SB // P)], axis=1)
    return np.ascontiguousarray(s.astype(ml_dtypes.bfloat16))


def kernel(query, key, value, mask, Wq, bq, Wk, bk, Wv, bv, Wo, bo):
    query = np.asarray(query, dtype=np.float32)
    key = np.asarray(key, dtype=np.float32)
    value = np.asarray(value, dtype=np.float32)
    m2d = np.asarray(mask).reshape(np.asarray(mask).shape[-2:])
    mask_mode = _detect_mask_mode(m2d)

    nc = _get(mask_mode)

    xq = [_bf16(query[b]) for b in range(B)]
    xk = [_bf16(key[b]) for b in range(B)]
    xv = [_bf16(value[b]) for b in range(B)]

    def prep_w(W, g):     # rows [256g, 256g+256) of W, transposed -> [128, 8, 256]
        sl = np.asarray(W, np.float32)[g * C:(g + 1) * C, :].T
        return _bf16(sl.reshape(DK, P, C).transpose(1, 0, 2))

    def prep_wo(g):       # Wo[:, 256g:256g+256].T -> [128, 2, 1024]
        sl = np.asarray(Wo, np.float32)[:, g * C:(g + 1) * C].T
        return _bf16(sl.reshape(C // P, P, D).transpose(1, 0, 2))

    def prep_b(b_, g):
        sl = np.asarray(b_, np.float32)[g * C:(g + 1) * C]
        return np.ascontiguousarray(sl.reshape(C // P, P).T)

    def prep_bvb(g):
        sl = np.asarray(bv, np.float32)[g * C:(g + 1) * C]
        return np.ascontiguousarray(np.broadcast_to(sl[None, :], (P, C)))

    strips = _strips() if mask_mode == "causal" else None
    maskT = _bf16(m2d.T.astype(np.float32)) if mask_mode == "generic" else None

    in_maps = []
    for c in range(NCORES):
        b, g = c // GROUPS, c % GROUPS
        im = dict(
            xq=xq[b], xk=xk[b], xv=xv[b],
            wqT=prep_w(Wq, g), wkT=prep_w(Wk, g), wvT=prep_w(Wv, g),
            woT=prep_wo(g), bq=prep_b(bq, g), bk=prep_b(bk, g),
            bvb=prep_bvb(g),
        )
        if strips is not None:
            im["strips"] = strips
        if maskT is not None:
            im["maskT"] = maskT
        in_maps.append(im)

    res = run_bass_kernel_spmd(nc, in_maps, list(range(NCORES)))
    partials = np.stack([res.results[c]["o"] for c in range(NCORES)])
    out = partials.reshape(B, GROUPS, S, D).sum(axis=1)
    out = out + np.asarray(bo, np.float32)[None, None, :]
    return out.astype(np.float32)
